# revision 19
# baseline (speedup 1.0000x reference)
"""DCompGCN layer on 8 TRN2 NeuronCores.

Sharding: edges partitioned by dst-window (6272 nodes/core, 49 windows of 128).
Zero-transpose dataflow: transposed dma_gathers give channel-major [feat, edge]
tiles; using the gathered tile as the matmul *stationary* operand yields
edge-major outputs; the segment-sum is a matmul (lhsT=edata_att, rhs=onehot)
accumulating channel-major window aggregates in PSUM. Projection + self-loop
are folded into one weight matrix applied once per 128-node window. BatchNorm
stats AllReduce across the 8 cores; tanh+affine fused in one ACT op.
"""
import numpy as np

V, E, IN, OUT, K, NREL = 50000, 640000, 128, 128, 4, 237
D = IN // K
HALF = E // 2
P = 128
NCORES = 8
WPC = 49                      # windows per core
VW = WPC * P                  # 6272 nodes per core
SUPT = 8                      # tiles per gather supertile
SUP = SUPT * P                # 1024 edges
TATT = 4                      # tiles per attention batch
H0_ROWS = 32768               # zero row + nodes 0..32766
H1_ROWS = 17234               # zero row + nodes 32767..49999


def _idx_layout(a):
    """int16 [L] -> [128, L//16] gather layout (i at [i%16, i//16]), x8 replicated."""
    L = a.shape[0]
    return np.ascontiguousarray(np.tile(a.reshape(L // 16, 16).T, (8, 1)))


def _prep(node_repr, rel_repr, src, dst, etype, norm):
    """Per-core edge streams, padded so all 8 cores share one static schedule."""
    core = dst // VW
    h_flag = (np.arange(E) >= HALF).astype(np.int64)
    w_of = (dst - core * VW) // P
    # group edges by (core, half, window)
    key = (core * 2 + h_flag) * WPC + w_of
    order = np.argsort(key, kind="stable")
    counts = np.bincount(key, minlength=NCORES * 2 * WPC).reshape(NCORES, 2, WPC)
    tiles = (counts + P - 1) // P
    T_hw = tiles.max(axis=0)                       # [2, WPC] static tile counts
    T_hw = np.maximum(T_hw, 1)
    NT = int(T_hw.sum())
    NT = ((NT + SUPT - 1) // SUPT) * SUPT          # pad to supertile multiple
    extra = NT - int(T_hw.sum())
    T_hw[1, WPC - 1] += extra
    L = NT * P

    # tile metadata (same for all cores): (h, w, first, last)
    tile_meta = []
    for h in range(2):
        for w in range(WPC):
            n = int(T_hw[h, w])
            for t in range(n):
                tile_meta.append((h, w, t == 0, t == n - 1))
    cum = np.concatenate([[0], np.cumsum(counts.ravel())])

    cores = []
    for c in range(NCORES):
        i0 = np.zeros(L, np.int16)
        i1 = np.zeros(L, np.int16)
        iet = np.zeros(L, np.int16)
        idl = np.zeros(L, np.int16)
        nrm = np.zeros(L, np.float32)
        oh = np.zeros((P, L), np.float16)
        base = 0
        for h in range(2):
            for w in range(WPC):
                cell = (c * 2 + h) * WPC + w
                eidx = order[cum[cell]:cum[cell + 1]]
                n = eidx.size
                pos = base + np.arange(n)
                sg = src[eidx]
                i0[pos] = np.where(sg <= 32766, sg + 1, 0).astype(np.int16)
                i1[pos] = np.where(sg >= 32767, sg - 32766, 0).astype(np.int16)
                iet[pos] = etype[eidx].astype(np.int16)
                idl[pos] = (dst[eidx] - c * VW).astype(np.int16)
                nrm[pos] = norm[eidx]
                oh[pos % P, (pos // P) * P + (dst[eidx] - c * VW - w * P)] = 1.0
                base += int(T_hw[h, w]) * P
        wt = np.zeros((VW, P), np.float16)
        lo, hi = c * VW, min((c + 1) * VW, V)
        wt[: hi - lo] = node_repr[lo:hi].astype(np.float16)
        cores.append(dict(
            I0=_idx_layout(i0), I1=_idx_layout(i1), IET=_idx_layout(iet),
            IDL=_idx_layout(idl),
            NORMT=np.ascontiguousarray(nrm.reshape(NT, P).T),
            OH=oh, WT=wt,
        ))
    return cores, tile_meta, NT, L


def _build(NT, tile_meta):
    import concourse.tile as tile
    from concourse import bacc, mybir

    F16, F32, I16 = mybir.dt.float16, mybir.dt.float32, mybir.dt.int16
    AX = mybir.AxisListType.X
    OP = mybir.AluOpType
    AF = mybir.ActivationFunctionType
    L = NT * P
    NS = NT // SUPT

    nc = bacc.Bacc(None, target_bir_lowering=False, debug=True)
    pr = nc.declare_dram_parameter
    H0 = pr("H0", [H0_ROWS, P], F16, isOutput=False)
    H1 = pr("H1", [H1_ROWS, P], F16, isOutput=False)
    WT = pr("WT", [VW, P], F16, isOutput=False)
    RT = pr("RT", [512, P], F16, isOutput=False)
    I0 = pr("I0", [P, L // 16], I16, isOutput=False)
    I1 = pr("I1", [P, L // 16], I16, isOutput=False)
    IET = pr("IET", [P, L // 16], I16, isOutput=False)
    NORMT = pr("NORMT", [P, NT], F32, isOutput=False)
    OH = pr("OH", [P, L], F16, isOutput=False)
    W1 = pr("W1", [P, P], F16, isOutput=False)
    A1 = pr("A1", [P, 4], F16, isOutput=False)
    A2 = pr("A2", [P, 4], F16, isOutput=False)
    WINF = pr("WINF", [P, P], F32, isOutput=False)
    WOUTF = pr("WOUTF", [P, P], F32, isOutput=False)
    RELT = pr("RELT", [P, 474], F32, isOutput=False)
    WREL = pr("WREL", [P, P], F32, isOutput=False)
    GAM = pr("GAM", [P, 1], F32, isOutput=False)
    BET = pr("BET", [P, 1], F32, isOutput=False)
    HOUT = pr("HOUT", [P, VW], F32, isOutput=True)
    RELOUT = pr("RELOUT", [P, 474], F32, isOutput=True)

    bn_in = nc.dram_tensor("bn_in", [P, 2], F32)
    bn_out = nc.dram_tensor("bn_out", [P, 2], F32, addr_space="Shared")

    with tile.TileContext(nc) as tc:
        with tc.tile_pool(name="const", bufs=1) as cp, \
             tc.tile_pool(name="pers", bufs=1) as pp, \
             tc.tile_pool(name="io", bufs=2) as io, \
             tc.tile_pool(name="tl", bufs=3) as tl, \
             tc.tile_pool(name="psE", bufs=2, space="PSUM") as psE, \
             tc.tile_pool(name="psL", bufs=1, space="PSUM") as psL, \
             tc.tile_pool(name="psT", bufs=2, space="PSUM") as psT, \
             tc.tile_pool(name="psA", bufs=1, space="PSUM") as psA, \
             tc.tile_pool(name="psH", bufs=1, space="PSUM") as psH:

            def ld(param, shape, dt, tag):
                t = cp.tile(shape, dt, tag=tag)
                nc.sync.dma_start(out=t[:], in_=param[:])
                return t

            w1_sb = ld(W1, [P, P], F16, "w1c")
            a1_sb = ld(A1, [P, 4], F16, "a1c")
            a2_sb = ld(A2, [P, 4], F16, "a2c")
            winf_sb = ld(WINF, [P, P], F32, "winfc")
            woutf_sb = ld(WOUTF, [P, P], F32, "woutfc")
            relt_sb = ld(RELT, [P, 474], F32, "reltc")
            wrel_sb = ld(WREL, [P, P], F32, "wrelc")
            gam_sb = ld(GAM, [P, 1], F32, "gamc")
            bet_sb = ld(BET, [P, 1], F32, "betc")
            normt_sb = ld(NORMT, [P, NT], F32, "normtc")

            from concourse.masks import make_identity
            ident = cp.tile([P, P], F16, tag="identc")
            make_identity(nc, ident[:])

            # per-window q2 = WT_win @ A2  (for dst-side attention logits)
            q2all = pp.tile([P, WPC * 4], F16)
            for w in range(WPC):
                wtile = io.tile([P, P], F16, tag="wtt")
                nc.sync.dma_start(out=wtile[:], in_=WT[w * P:(w + 1) * P, :])
                wtp = psT.tile([P, 2 * P], F16, space="PSUM", tag="tps")
                nc.tensor.transpose(out=wtp[:, :P], in_=wtile[:],
                                    identity=ident[:])
                wtt_sb = io.tile([P, P], F16, tag="wtts")
                nc.scalar.copy(out=wtt_sb[:], in_=wtp[:, :P])
                q2ps = psL.tile([P, TATT * 4], F32, space="PSUM", tag="lgb")
                nc.tensor.matmul(out=q2ps[:, :4], lhsT=wtt_sb[:], rhs=a2_sb[:],
                                 start=True, stop=True, skip_group_check=True)
                nc.scalar.copy(out=q2all[:, w * 4:(w + 1) * 4], in_=q2ps[:, :4])

            aggin_sb = pp.tile([P, WPC * P], F32)
            h_sb = pp.tile([P, WPC * P], F32)
            ssum = pp.tile([P, WPC], F32)
            ssq = pp.tile([P, WPC], F32)

            # rel_out = w_rel.T-applied: [och, r] = sum_i w_rel[i,och] relT[i,r]
            rel_ps = psH.tile([P, 474], F32, space="PSUM", tag="hps")
            nc.tensor.matmul(out=rel_ps[:], lhsT=wrel_sb[:], rhs=relt_sb[:],
                             start=True, stop=True)
            rel_o = io.tile([P, 474], F32)
            nc.scalar.copy(out=rel_o[:], in_=rel_ps[:])
            nc.sync.dma_start(out=RELOUT[:], in_=rel_o[:])

            agg_tile = [None, None]
            for s in range(NS):
                cs, ce = s * (SUP // 16), (s + 1) * (SUP // 16)  # idx cols
                i0_sb = tl.tile([P, SUP // 16], I16, tag="i0")
                nc.sync.dma_start(out=i0_sb[:], in_=I0[:, cs:ce])
                i1_sb = tl.tile([P, SUP // 16], I16, tag="i1")
                nc.sync.dma_start(out=i1_sb[:], in_=I1[:, cs:ce])
                iet_sb = tl.tile([P, SUP // 16], I16, tag="iet")
                nc.sync.dma_start(out=iet_sb[:], in_=IET[:, cs:ce])
                oh_sb = tl.tile([P, SUP], F16, tag="oh")
                nc.sync.dma_start(out=oh_sb[:], in_=OH[:, s * SUP:(s + 1) * SUP])

                def gath(tab, idx_sb, tag):
                    g = tl.tile([P, SUPT, P], F16, tag=tag)
                    nc.gpsimd.dma_gather(
                        out_ap=g[:], in_ap=tab[:], idxs_ap=idx_sb[:],
                        num_idxs=SUP, num_idxs_reg=SUP, elem_size=P, transpose=False)
                    return g
                g0 = gath(H0, i0_sb, "g0")
                g1 = gath(H1, i1_sb, "g1")
                gr = gath(RT, iet_sb, "gr")

                ect = tl.tile([P, SUPT, P], F16, tag="ect")
                nc.vector.tensor_tensor(out=ect[:], in0=g0[:], in1=g1[:], op=OP.add)
                nc.vector.tensor_tensor(out=ect[:], in0=ect[:], in1=gr[:], op=OP.mult)

                for b in range(SUPT // TATT):
                    pse = psE.tile([P, TATT * P], F32, space="PSUM")
                    lgb = psL.tile([P, TATT * 4], F32, space="PSUM")
                    tb = s * SUPT + b * TATT          # global tile idx of batch
                    for j in range(TATT):
                        jj = b * TATT + j             # tile within supertile
                        h, w, first, last = tile_meta[s * SUPT + jj]
                        tps = psT.tile([P, 2 * P], F16, space="PSUM", tag="tps")
                        nc.tensor.transpose(out=tps[:, :P], in_=ect[:, jj, :],
                                            identity=ident[:])
                        nc.tensor.transpose(out=tps[:, P:],
                                            in_=oh_sb[:, jj * P:(jj + 1) * P],
                                            identity=ident[:])
                        ect_j = tl.tile([P, P], F16, tag="ectj")
                        nc.scalar.copy(out=ect_j[:], in_=tps[:, :P])
                        oht_j = tl.tile([P, P], F16, tag="ohtj")
                        nc.vector.tensor_copy(out=oht_j[:], in_=tps[:, P:])
                        nc.tensor.matmul(out=pse[:, j * P:(j + 1) * P], lhsT=ect_j[:],
                                         rhs=w1_sb[:], start=True, stop=True,
                                         skip_group_check=True)
                        nc.tensor.matmul(out=lgb[:, j * 4:(j + 1) * 4], lhsT=ect_j[:],
                                         rhs=a1_sb[:], start=True, stop=False,
                                         skip_group_check=True)
                        nc.tensor.matmul(out=lgb[:, j * 4:(j + 1) * 4], lhsT=oht_j[:],
                                         rhs=q2all[:, w * 4:(w + 1) * 4],
                                         start=False, stop=True,
                                         skip_group_check=True)
                    # attention batch: [128e, TATT*4]
                    expb = tl.tile([P, TATT * 4], F32, tag="expb")
                    nc.scalar.activation(out=expb[:], in_=lgb[:], func=AF.Relu)
                    emax = tl.tile([P, TATT * 4], F32, tag="emax")
                    nc.scalar.activation(out=emax[:], in_=expb[:], func=AF.Exp)
                    se = tl.tile([P, TATT], F32, tag="se")
                    nc.vector.tensor_reduce(
                        out=se[:], in_=emax[:].rearrange("p (t k) -> p t k", k=4),
                        axis=AX, op=OP.add)
                    rc = tl.tile([P, TATT], F32, tag="rc")
                    nc.vector.reciprocal(out=rc[:], in_=se[:])
                    sc = tl.tile([P, TATT], F32, tag="sc")
                    nc.vector.tensor_tensor(out=sc[:], in0=rc[:],
                                            in1=normt_sb[:, tb:tb + TATT], op=OP.mult)
                    attw = tl.tile([P, TATT * 4], F32, tag="attw")
                    nc.vector.tensor_tensor(
                        out=attw[:].rearrange("p (t k) -> p t k", k=4),
                        in0=emax[:].rearrange("p (t k) -> p t k", k=4),
                        in1=sc[:].unsqueeze(2).to_broadcast([P, TATT, 4]),
                        op=OP.mult)
                    for j in range(TATT):
                        jj = b * TATT + j
                        t = tb + j
                        h, w, first, last = tile_meta[t]
                        if first:
                            agg_new = psA.tile([P, P], F32, space="PSUM",
                                               tag=f"agg{h}")
                            agg_tile[h] = agg_new
                        ea = tl.tile([P, P], F16, tag="ea")
                        nc.vector.tensor_tensor(
                            out=ea[:].rearrange("p (k d) -> p k d", k=4),
                            in0=pse[:, j * P:(j + 1) * P].rearrange(
                                "p (k d) -> p k d", k=4),
                            in1=attw[:, j * 4:(j + 1) * 4].unsqueeze(2)
                                .to_broadcast([P, 4, D]),
                            op=OP.mult)
                        nc.tensor.matmul(out=agg_tile[h][:], lhsT=ea[:],
                                         rhs=oh_sb[:, jj * P:(jj + 1) * P],
                                         start=first, stop=last,
                                         skip_group_check=True)
                        if last:
                            wsl = slice(w * P, (w + 1) * P)
                            if h == 0:
                                nc.scalar.copy(out=aggin_sb[:, wsl],
                                               in_=agg_tile[0][:])
                            else:
                                aggo = tl.tile([P, P], F32, tag="aggo")
                                nc.scalar.copy(out=aggo[:], in_=agg_tile[1][:])
                                hps = psH.tile([P, P], F32, space="PSUM", tag="hps")
                                nc.tensor.matmul(out=hps[:], lhsT=winf_sb[:],
                                                 rhs=aggin_sb[:, wsl],
                                                 start=True, stop=False,
                                                 skip_group_check=True)
                                nc.tensor.matmul(out=hps[:], lhsT=woutf_sb[:],
                                                 rhs=aggo[:],
                                                 start=False, stop=True,
                                                 skip_group_check=True)
                                nc.scalar.copy(out=h_sb[:, wsl], in_=hps[:])
                                sq = tl.tile([P, P], F32, tag="sq")
                                nc.scalar.square(out=sq[:], in_=hps[:])
                                nc.vector.tensor_reduce(
                                    out=ssum[:, w:w + 1], in_=h_sb[:, wsl],
                                    axis=AX, op=OP.add)
                                nc.vector.tensor_reduce(
                                    out=ssq[:, w:w + 1], in_=sq[:],
                                    axis=AX, op=OP.add)

            # ---- BatchNorm finalize ----
            tot = io.tile([P, 2], F32)
            nc.vector.tensor_reduce(out=tot[:, 0:1], in_=ssum[:], axis=AX, op=OP.add)
            nc.vector.tensor_reduce(out=tot[:, 1:2], in_=ssq[:], axis=AX, op=OP.add)
            nc.sync.dma_start(out=bn_in[:], in_=tot[:])
            nc.gpsimd.collective_compute(
                "AllReduce", OP.add, replica_groups=[list(range(NCORES))],
                ins=[bn_in[:]], outs=[bn_out[:]])
            bn_sb = io.tile([P, 2], F32)
            nc.sync.dma_start(out=bn_sb[:], in_=bn_out[:])
            mean = io.tile([P, 1], F32)
            nc.vector.tensor_scalar_mul(mean[:], bn_sb[:, 0:1], 1.0 / V)
            ex2 = io.tile([P, 1], F32)
            nc.vector.tensor_scalar_mul(ex2[:], bn_sb[:, 1:2], 1.0 / V)
            m2 = io.tile([P, 1], F32)
            nc.scalar.square(out=m2[:], in_=mean[:])
            var = io.tile([P, 1], F32)
            nc.vector.tensor_tensor(out=var[:], in0=ex2[:], in1=m2[:], op=OP.subtract)
            nc.vector.tensor_scalar_add(var[:], var[:], 1e-5)
            sd = io.tile([P, 1], F32)
            nc.scalar.sqrt(out=sd[:], in_=var[:])
            rstd = io.tile([P, 1], F32)
            nc.vector.reciprocal(out=rstd[:], in_=sd[:])
            scal = io.tile([P, 1], F32)
            nc.vector.tensor_tensor(out=scal[:], in0=rstd[:], in1=gam_sb[:], op=OP.mult)
            msc = io.tile([P, 1], F32)
            nc.vector.tensor_tensor(out=msc[:], in0=mean[:], in1=scal[:], op=OP.mult)
            shift = io.tile([P, 1], F32)
            nc.vector.tensor_tensor(out=shift[:], in0=bet_sb[:], in1=msc[:],
                                    op=OP.subtract)
            for w in range(WPC):
                wsl = slice(w * P, (w + 1) * P)
                ot = tl.tile([P, P], F32, tag="ot")
                nc.scalar.activation(out=ot[:], in_=h_sb[:, wsl], func=AF.Tanh,
                                     bias=shift[:, 0:1], scale=scal[:, 0:1])
                nc.sync.dma_start(out=HOUT[:, wsl], in_=ot[:])

    nc.compile()
    return nc


def kernel(node_repr, rel_repr, src, dst, etype, norm,
           node_w, node_rel_w, in_w, out_w, att_w,
           loop_rel, loop_w, w_rel, bias, bn_gamma, bn_beta):
    from concourse.bass_utils import run_bass_kernel_spmd

    node_repr = np.asarray(node_repr, np.float32)
    rel_repr = np.asarray(rel_repr, np.float32)
    src = np.asarray(src); dst = np.asarray(dst); etype = np.asarray(etype)
    norm = np.asarray(norm, np.float32)
    node_w = np.asarray(node_w, np.float64)
    node_rel_w = np.asarray(node_rel_w, np.float64)
    in_w = np.asarray(in_w, np.float64); out_w = np.asarray(out_w, np.float64)
    att_w = np.asarray(att_w, np.float64)
    loop_rel = np.asarray(loop_rel, np.float64)
    loop_w = np.asarray(loop_w, np.float64)

    cores, tile_meta, NT, L = _prep(node_repr, rel_repr,
                                    np.asarray(src, np.int64),
                                    np.asarray(dst, np.int64),
                                    np.asarray(etype, np.int64), norm)

    # weight folds
    W1v = np.transpose(node_rel_w, (1, 0, 2)).reshape(P, P)         # [i, k*D+d]
    A1v = np.einsum("kid,d->ik", node_rel_w, att_w[:D, 0])
    A2v = np.einsum("kid,d->ik", node_w, att_w[D:, 0])
    Wb_in = np.zeros((P, P)); Wb_out = np.zeros((P, P))
    for k in range(K):
        Wb_in[k * D:(k + 1) * D, k * D:(k + 1) * D] = in_w[k]
        Wb_out[k * D:(k + 1) * D, k * D:(k + 1) * D] = out_w[k]
    M = np.eye(P) + (loop_rel[0][:, None] * loop_w) / 3.0
    WINFv = (Wb_in / 3.0) @ M
    WOUTFv = (Wb_out / 3.0) @ M

    H0v = np.zeros((H0_ROWS, P), np.float16)
    H0v[1:32768] = node_repr[0:32767].astype(np.float16)
    H1v = np.zeros((H1_ROWS, P), np.float16)
    H1v[1:] = node_repr[32767:V].astype(np.float16)
    RTv = np.zeros((512, P), np.float16)
    RTv[:2 * NREL] = rel_repr.astype(np.float16)

    shared = dict(
        H0=H0v, H1=H1v, RT=RTv,
        W1=W1v.astype(np.float16), A1=A1v.astype(np.float16),
        A2=A2v.astype(np.float16),
        WINF=WINFv.astype(np.float32), WOUTF=WOUTFv.astype(np.float32),
        RELT=np.ascontiguousarray(rel_repr.T.astype(np.float32)),
        WREL=np.asarray(w_rel, np.float32),
        GAM=np.asarray(bn_gamma, np.float32).reshape(P, 1),
        BET=np.asarray(bn_beta, np.float32).reshape(P, 1),
    )
    in_maps = [dict(shared, **cores[c]) for c in range(NCORES)]

    import os, time
    nc = _build(NT, tile_meta)
    res = run_bass_kernel_spmd(nc, in_maps, list(range(NCORES)))
    if int(os.environ.get("KERNEL_TIME", "0")):
        _timed_exec(nc, in_maps)

    hslices = [res.results[c]["HOUT"].T for c in range(NCORES)]      # [6272,128]
    node_out = np.concatenate(hslices, axis=0)[:V].astype(np.float32)
    rel_out = np.ascontiguousarray(res.results[0]["RELOUT"].T.astype(np.float32))
    return node_out, rel_out


def _timed_exec(nc, in_maps):
    """Re-run the compiled program on device, timing only dispatch+execute."""
    import time
    import jax
    import jax.numpy as jnp
    from concourse import bass2jax, mybir
    from jax.sharding import Mesh, PartitionSpec
    from jax.experimental.shard_map import shard_map

    bass2jax.install_neuronx_cc_hook()
    partition_name = (nc.partition_id_tensor.name
                      if nc.partition_id_tensor else None)
    in_names, out_names, out_avals, zero_outs = [], [], [], []
    for alloc in nc.m.functions[0].allocations:
        if not isinstance(alloc, mybir.MemoryLocationSet):
            continue
        name = alloc.memorylocations[0].name
        if alloc.kind == "ExternalInput":
            if name != partition_name:
                in_names.append(name)
        elif alloc.kind == "ExternalOutput":
            shape = tuple(alloc.tensor_shape)
            dtype = mybir.dt.np(alloc.dtype)
            out_names.append(name)
            out_avals.append(jax.core.ShapedArray(shape, dtype))
            zero_outs.append(np.zeros(shape, dtype))
    n_params = len(in_names)
    all_names = list(in_names) + list(out_names)
    if partition_name is not None:
        all_names.append(partition_name)

    def _body(*args):
        operands = list(args)
        if partition_name is not None:
            operands.append(bass2jax.partition_id_tensor())
        return tuple(bass2jax._bass_exec_p.bind(
            *operands, out_avals=tuple(out_avals), in_names=tuple(all_names),
            out_names=tuple(out_names), lowering_input_output_aliases=(),
            sim_require_finite=True, sim_require_nnan=True, nc=nc))

    devices = jax.devices()[:NCORES]
    mesh = Mesh(np.asarray(devices), ("core",))
    nin = n_params + len(out_names)
    sharded = jax.jit(
        shard_map(_body, mesh=mesh, in_specs=(PartitionSpec("core"),) * nin,
                  out_specs=(PartitionSpec("core"),) * len(out_names),
                  check_rep=False),
        keep_unused=True)
    if nc.dbg_addr is not None:
        in_maps = [{**m, nc.dbg_addr.name: np.zeros((1, 2), np.uint32)}
                   for m in in_maps]
    concat_in = [np.concatenate([in_maps[c][k] for c in range(NCORES)], axis=0)
                 for k in in_names]
    concat_zero = [np.zeros((NCORES * z.shape[0], *z.shape[1:]), z.dtype)
                   for z in zero_outs]
    put = [jax.device_put(a) for a in concat_in + concat_zero]
    r = sharded(*put)
    jax.block_until_ready(r)
    times = []
    for _ in range(3):
        t0 = time.perf_counter()
        r = sharded(*put)
        jax.block_until_ready(r)
        times.append(time.perf_counter() - t0)
    dt = min(times)
    print(f"HW exec time: {dt * 1e9:.0f} ns")
